# revision 9
# baseline (speedup 1.0000x reference)
"""Trainium2 Bass kernel for the Lorentz (hyperboloid) embedding loss.

Data-parallel over the batch: B=16384 anchors are sharded 2048-per-core
across 8 NeuronCores. Per anchor the kernel needs the anchor row plus its
50 candidate rows of the 1M x 32 fp32 table. The embedding-row
indirection is resolved on the host into a densely packed per-core
operand (the container's compile path mis-lowers every indirect/gather
DMA primitive).

The packed operand is bf16 with an alpha-transform that keeps the
numerics safe: x0 ~= 1 + 5e-6 would collapse to 1.0 in bf16, so rows are
re-centered. Candidate rows are packed as [x0-1, s_1..s_31], anchor rows
as [1.0, -s_1..s_31]. The elementwise product then satisfies
  sum_d m[d] = beta_k - dot(s_i, s_k)
and  y = d - 1 = alpha_i + beta_k - dot   (+ negligible alpha*beta)
with alpha_i added back per-tile on the ScalarE (activation bias).
bf16 halves HBM traffic and lets the DVE run tensor_tensor at 2x.

Engine split per group: DVE does the multiply and the first two
tree-add stages (2x bf16, in-place in m to minimize tile/semaphore
count -- the pre/postamble cost scales with semaphore count); GpSimd
does the last three tree stages and the row-sum reduce; ScalarE fuses
alpha-add + clamp into one Relu activation per tile
(a1 = relu(ys + alpha - 1e-6), with alpha-1e-6 host-packed), then
Square/Sqrt for arcosh: t = (a1 + 1+1e-6) + sqrt((a1 + 1+1e-6)^2 - 1).
The DVE-side consumers of ScalarE results run one group late so the
DVE queue never blocks. loss = ln(t0 * (sum 1/t + 1e-6)).
Groups are sized [2,4,4,4,2] for a fast pipeline ramp and a short
serial tail; the output store is split in two.
"""
import os
import sys

for _p in ("/opt/trn_rl_repo", "/root/.axon_site/_ro/trn_rl_repo"):
    if _p not in sys.path and os.path.isdir(_p):
        sys.path.append(_p)

import numpy as np

N_ITEMS_P1 = 1_000_001
DIM = 32
B = 16384
N_KS = 50
W = N_KS + 1          # rows per anchor: anchor + 50 candidates
P = 128               # SBUF partitions = anchors per tile
N_CORES = 8
B_SHARD = B // N_CORES
N_TILES = B_SHARD // P

GROUP_TILES = [2, 4, 4, 4, 2]     # tiles per reduction group
assert sum(GROUP_TILES) == N_TILES
GROUP_START = [sum(GROUP_TILES[:i]) for i in range(len(GROUP_TILES))]
N_GRP = len(GROUP_TILES)
CLAMP1 = float(np.float32(1.0 + 1e-6))

_nc_cache = None


def _build():
    import concourse.bacc as bacc
    import concourse.tile as tile
    from concourse import mybir

    F32 = mybir.dt.float32
    BF16 = mybir.dt.bfloat16
    AF = mybir.ActivationFunctionType
    OP = mybir.AluOpType

    nc = bacc.Bacc(
        "TRN2", target_bir_lowering=False, debug=False, num_devices=N_CORES
    )
    # g[b, 0, :] = [1, -s_i]; g[b, 1+n, :] = [beta_kn, s_kn]  (host-packed bf16)
    g_in = nc.declare_dram_parameter("g", [B_SHARD, W * DIM], BF16, isOutput=False)
    # alpha[p, t] = x0(anchor t*128+p) - 1 - 1e-6, fp32 (clamp folded in)
    a_in = nc.declare_dram_parameter("alpha", [P, N_TILES], F32, isOutput=False)
    loss = nc.declare_dram_parameter("loss", [B_SHARD], F32, isOutput=True)

    from concourse.masks import make_identity

    with tile.TileContext(nc) as tc:
        with (
            tc.tile_pool(name="cons", bufs=1) as cons,
            tc.tile_pool(name="big", bufs=8) as big,
            tc.tile_pool(name="mid", bufs=2) as mid,
            tc.tile_pool(name="small", bufs=2) as small,
            tc.tile_pool(name="psum", bufs=2, space="PSUM") as psum,
        ):
            g_tiles = {}
            n_load = 0
            # (group, first-tile-in-group, tiles-per-load)
            load_plan = []
            for gi, gt in enumerate(GROUP_TILES):
                if gi == 0:
                    load_plan.append([(0, 1), (1, 1)][:gt])
                elif gt == 2:
                    load_plan.append([(0, 2)])
                else:
                    load_plan.append([(0, 2), (2, 2)])

            def issue_load(gi, tg, tpi):
                nonlocal n_load
                t = GROUP_START[gi] + tg
                g = big.tile([P, tpi, W * DIM], BF16, tag="g")
                src = g_in[t * P:(t + tpi) * P, :].rearrange(
                    "(c p) f -> p c f", p=P
                )
                eng = nc.sync if n_load % 2 == 0 else nc.scalar
                eng.dma_start(out=g[:], in_=src)
                n_load += 1
                g_tiles[(gi, tg)] = g

            for tg, tpi in load_plan[0]:
                issue_load(0, tg, tpi)
            for tg, tpi in load_plan[1]:
                issue_load(1, tg, tpi)

            alpha_sb = cons.tile([P, N_TILES], F32)
            nc.sync.dma_start(out=alpha_sb[:], in_=a_in[:, :])

            ident = cons.tile([P, P], F32)
            make_identity(nc, ident[:])
            bias_neg1 = cons.tile([P, 1], F32)
            nc.vector.memset(bias_neg1[:], -1.0)
            bias_c1 = cons.tile([P, 1], F32)
            nc.vector.memset(bias_c1[:], CLAMP1)
            # preload the Ln table set early so the final Ln doesn't pay it
            warm = cons.tile([P, 1], F32)
            nc.scalar.activation(out=warm[:], in_=bias_c1[:], func=AF.Ln)

            t_all = cons.tile([P, N_TILES, N_KS], F32)
            s1 = cons.tile([P, N_TILES], F32)
            lv_all = cons.tile([P, N_TILES], F32)

            a1_t = {}
            r_t = {}

            def group_front(gi):
                """Multiply + tree stages 1-2 on DVE, stages 3-5 + reduce prep
                on GpSimd, alpha/clamp/square/sqrt on ScalarE."""
                gt = GROUP_TILES[gi]
                t0 = GROUP_START[gi]
                m = mid.tile([P, gt, N_KS, DIM], BF16, tag=f"m{gt}")
                for tg, tpi in load_plan[gi]:
                    g = g_tiles.pop((gi, tg))
                    g4 = g[:].rearrange("p c (w d) -> p c w d", d=DIM)
                    nc.vector.tensor_tensor(
                        out=m[:, tg:tg + tpi],
                        in0=g4[:, :, 1:, :],
                        in1=g4[:, :, 0:1, :].to_broadcast([P, tpi, N_KS, DIM]),
                        op=OP.mult,
                    )
                # binary-tree reduce over d, in place in m
                nc.vector.tensor_tensor(
                    out=m[:, :, :, 0:16], in0=m[:, :, :, 0:16],
                    in1=m[:, :, :, 16:32], op=OP.add,
                )
                nc.vector.tensor_tensor(
                    out=m[:, :, :, 0:8], in0=m[:, :, :, 0:8],
                    in1=m[:, :, :, 8:16], op=OP.add,
                )
                nc.gpsimd.tensor_tensor(
                    out=m[:, :, :, 0:4], in0=m[:, :, :, 0:4],
                    in1=m[:, :, :, 4:8], op=OP.add,
                )
                nc.gpsimd.tensor_tensor(
                    out=m[:, :, :, 0:2], in0=m[:, :, :, 0:2],
                    in1=m[:, :, :, 2:4], op=OP.add,
                )
                ys = small.tile([P, gt, N_KS], F32, tag="ys")
                nc.gpsimd.tensor_tensor(
                    out=ys[:], in0=m[:, :, :, 0], in1=m[:, :, :, 1], op=OP.add,
                )
                # a1 = relu(ys + alpha - 1e-6); later t = (a1 + 1+1e-6) + r
                # handles the reference clamp d<=1 -> 1+1e-6 up to the
                # measure-zero band y in (0, 1e-6).
                a1 = small.tile([P, gt, N_KS], F32, tag="a1")
                for c in range(gt):
                    nc.scalar.activation(
                        out=a1[:, c], in_=ys[:, c], func=AF.Relu,
                        bias=alpha_sb[:, t0 + c:t0 + c + 1],
                    )
                sq = small.tile([P, gt, N_KS], F32, tag="sq")
                nc.scalar.activation(
                    out=sq[:], in_=a1[:], func=AF.Square, bias=bias_c1[:]
                )
                r = small.tile([P, gt, N_KS], F32, tag="r")
                nc.scalar.activation(
                    out=r[:], in_=sq[:], func=AF.Sqrt, bias=bias_neg1[:]
                )
                a1_t[gi] = a1
                r_t[gi] = r

            def group_back(gi):
                """t = (a1 + 1+1e-6) + r;  1/t;  row-sum."""
                gt = GROUP_TILES[gi]
                t0 = GROUP_START[gi]
                tg_ = t_all[:, t0:t0 + gt]
                nc.vector.scalar_tensor_tensor(
                    out=tg_, in0=a1_t.pop(gi)[:], scalar=CLAMP1,
                    in1=r_t.pop(gi)[:], op0=OP.add, op1=OP.add,
                )
                rec = small.tile([P, gt, N_KS], F32, tag="rec")
                nc.vector.reciprocal_approx_fast(out=rec[:].opt(), in_=tg_.opt())
                nc.vector.tensor_reduce(
                    out=s1[:, t0:t0 + gt], in_=rec[:],
                    axis=mybir.AxisListType.X, op=OP.add,
                )

            def endgame(lo, hi, part):
                """loss[lo:hi] = ln(t0 * (s1 + 1e-6)); transpose + store."""
                n = hi - lo
                nc.vector.scalar_tensor_tensor(
                    out=s1[:, lo:hi], in0=s1[:, lo:hi], scalar=1e-6,
                    in1=t_all[:, lo:hi, 0], op0=OP.add, op1=OP.mult,
                )
                nc.scalar.activation(
                    out=lv_all[:, lo:hi], in_=s1[:, lo:hi], func=AF.Ln
                )
                lv_t_ps = psum.tile([n, P], F32, space="PSUM", tag=f"ps{part}")
                nc.tensor.transpose(
                    out=lv_t_ps[:], in_=lv_all[:, lo:hi], identity=ident[:]
                )
                lv_t = cons.tile([n, P], F32, tag=f"lvt{part}")
                nc.vector.tensor_copy(out=lv_t[:], in_=lv_t_ps[:])
                nc.sync.dma_start(
                    out=loss[lo * P:hi * P].rearrange("(t p) -> t p", p=P),
                    in_=lv_t[:],
                )

            for gi in range(N_GRP):
                if gi + 2 < N_GRP:
                    for tg, tpi in load_plan[gi + 2]:
                        issue_load(gi + 2, tg, tpi)
                group_front(gi)
                if gi > 0:
                    group_back(gi - 1)
                if gi == N_GRP - 1:
                    endgame(0, GROUP_START[gi], 0)
            group_back(N_GRP - 1)
            endgame(GROUP_START[N_GRP - 1], N_TILES, 1)
    nc.compile()
    return nc


def _get_nc():
    global _nc_cache
    if _nc_cache is None:
        _nc_cache = _build()
    return _nc_cache


def _prep_in_maps(table, I, Ks):
    import ml_dtypes

    table = np.ascontiguousarray(np.asarray(table, dtype=np.float32))
    I = np.asarray(I).astype(np.int64)
    Ks = np.asarray(Ks).astype(np.int64)
    assert table.shape == (N_ITEMS_P1, DIM)
    assert I.shape == (B,) and Ks.shape == (B, N_KS)
    ik = np.concatenate([I[:, None], Ks], axis=1)       # [B, 51]
    rows = table[ik.reshape(-1)].reshape(B, W, DIM)     # [B, 51, 32] fp32
    pack = np.empty((B, W, DIM), dtype=ml_dtypes.bfloat16)
    pack[:, 1:, 0] = rows[:, 1:, 0] - 1.0               # beta_k
    pack[:, 1:, 1:] = rows[:, 1:, 1:]                   # s_k
    pack[:, 0, 0] = 1.0
    pack[:, 0, 1:] = -rows[:, 0, 1:]                    # -s_i
    # alpha - 1e-6: the clamp threshold is folded into the relu bias
    alpha = (rows[:, 0, 0] - 1.0 - 1e-6).astype(np.float32)
    g_full = pack.reshape(B, W * DIM)
    in_maps = []
    for c in range(N_CORES):
        sh = np.ascontiguousarray(g_full[c * B_SHARD:(c + 1) * B_SHARD])
        al = np.ascontiguousarray(
            alpha[c * B_SHARD:(c + 1) * B_SHARD].reshape(N_TILES, P).T
        )
        in_maps.append({"g": sh, "alpha": al})
    return in_maps


def _run(table, I, Ks, trace=False, **kwargs):
    from concourse.bass_utils import run_bass_kernel_spmd

    nc = _get_nc()
    in_maps = _prep_in_maps(table, I, Ks)
    res = run_bass_kernel_spmd(
        nc, in_maps, list(range(N_CORES)), trace=trace, **kwargs
    )
    out = np.concatenate(
        [np.asarray(res.results[c]["loss"]) for c in range(N_CORES)]
    ).astype(np.float32)
    return out, res


def kernel(table, I, Ks):
    out, _ = _run(table, I, Ks, trace=False)
    return out


# revision 10
# speedup vs baseline: 1.1663x; 1.1663x over previous
"""Trainium2 Bass kernel for the Lorentz (hyperboloid) embedding loss.

Data-parallel over the batch: B=16384 anchors are sharded 2048-per-core
across 8 NeuronCores. Per anchor the kernel needs the anchor row plus its
50 candidate rows of the 1M x 32 fp32 table. The embedding-row
indirection is resolved on the host into a densely packed per-core
operand (the container's compile path mis-lowers every indirect/gather
DMA primitive).

The packed operand is bf16 with an alpha-transform that keeps the
numerics safe: x0 ~= 1 + 5e-6 would collapse to 1.0 in bf16, so rows are
re-centered. Candidate rows are packed as [x0-1, s_1..s_31], anchor rows
as [1.0, -s_1..s_31]. The elementwise product then satisfies
  sum_d m[d] = beta_k - dot(s_i, s_k)
and  y = d - 1 = alpha_i + beta_k - dot   (+ negligible alpha*beta)
with alpha_i added back from a small fp32 side operand. bf16 halves HBM
traffic and lets the DVE run tensor_tensor at 2x; the d-reduction is a
binary tree of in-place tensor_tensor adds (2x) instead of
tensor_reduce (1x). Everything streams on DVE + ScalarE only: GpSimd
elementwise was measured 2-3x slower per op here AND its SBUF traffic
contends with the DVE's two-port reads, slowing both.

arcosh in y-space: ym = max(y,1e-6) (matches the reference clamp up to
the measure-zero band y in (0,1e-6)), t = (1+ym) + sqrt((1+ym)^2 - 1),
loss = ln(t0 * (sum 1/t + 1e-6)). ScalarE does Square/Sqrt/Ln (exactly
three activation table sets -- a fourth causes table thrashing); their
DVE-side consumers run one group late so the DVE queue never blocks on
ScalarE. Groups are sized [2,4,4,4,2] for fast pipeline ramp and short
serial tail; the output store is split in two. Tile/semaphore count is
kept low (in-place tree, few pool tags): the framework pre/postamble
cost scales with the number of semaphores.
"""
import os
import sys

for _p in ("/opt/trn_rl_repo", "/root/.axon_site/_ro/trn_rl_repo"):
    if _p not in sys.path and os.path.isdir(_p):
        sys.path.append(_p)

import numpy as np

N_ITEMS_P1 = 1_000_001
DIM = 32
B = 16384
N_KS = 50
W = N_KS + 1          # rows per anchor: anchor + 50 candidates
P = 128               # SBUF partitions = anchors per tile
N_CORES = 8
B_SHARD = B // N_CORES
N_TILES = B_SHARD // P

GROUP_TILES = [2, 4, 4, 4, 2]     # tiles per reduction group
assert sum(GROUP_TILES) == N_TILES
GROUP_START = [sum(GROUP_TILES[:i]) for i in range(len(GROUP_TILES))]
N_GRP = len(GROUP_TILES)
CLAMP1 = float(np.float32(1.0 + 1e-6))

_nc_cache = None


def _build():
    import concourse.bacc as bacc
    import concourse.tile as tile
    from concourse import mybir

    F32 = mybir.dt.float32
    BF16 = mybir.dt.bfloat16
    AF = mybir.ActivationFunctionType
    OP = mybir.AluOpType

    nc = bacc.Bacc(
        "TRN2", target_bir_lowering=False, debug=False, num_devices=N_CORES
    )
    # g[b, 0, :] = [1, -s_i]; g[b, 1+n, :] = [beta_kn, s_kn]  (host-packed bf16)
    g_in = nc.declare_dram_parameter("g", [B_SHARD, W * DIM], BF16, isOutput=False)
    # alpha[p, t] = x0(anchor t*128+p) - 1, fp32
    a_in = nc.declare_dram_parameter("alpha", [P, N_TILES], F32, isOutput=False)
    loss = nc.declare_dram_parameter("loss", [B_SHARD], F32, isOutput=True)

    from concourse.masks import make_identity

    with tile.TileContext(nc) as tc:
        with (
            tc.tile_pool(name="cons", bufs=1) as cons,
            tc.tile_pool(name="big", bufs=8) as big,
            tc.tile_pool(name="mid", bufs=2) as mid,
            tc.tile_pool(name="small", bufs=2) as small,
            tc.tile_pool(name="psum", bufs=2, space="PSUM") as psum,
        ):
            g_tiles = {}
            n_load = 0
            load_plan = []
            for gi, gt in enumerate(GROUP_TILES):
                if gi == 0:
                    load_plan.append([(0, 1), (1, 1)][:gt])
                elif gt == 2:
                    load_plan.append([(0, 2)])
                else:
                    load_plan.append([(0, 2), (2, 2)])

            def issue_load(gi, tg, tpi):
                nonlocal n_load
                t = GROUP_START[gi] + tg
                g = big.tile([P, tpi, W * DIM], BF16, tag="g")
                src = g_in[t * P:(t + tpi) * P, :].rearrange(
                    "(c p) f -> p c f", p=P
                )
                eng = nc.sync if n_load % 2 == 0 else nc.scalar
                eng.dma_start(out=g[:], in_=src)
                n_load += 1
                g_tiles[(gi, tg)] = g

            for tg, tpi in load_plan[0]:
                issue_load(0, tg, tpi)
            for tg, tpi in load_plan[1]:
                issue_load(1, tg, tpi)

            alpha_sb = cons.tile([P, N_TILES], F32)
            nc.sync.dma_start(out=alpha_sb[:], in_=a_in[:, :])

            ident = cons.tile([P, P], F32)
            make_identity(nc, ident[:])
            bias_neg1 = cons.tile([P, 1], F32)
            nc.vector.memset(bias_neg1[:], -1.0)
            bias_pos1 = cons.tile([P, 1], F32)
            nc.vector.memset(bias_pos1[:], 1.0)
            # preload the Ln table set early so the final Ln doesn't pay it
            warm = cons.tile([P, 1], F32)
            nc.scalar.activation(out=warm[:], in_=bias_pos1[:], func=AF.Ln)

            t_all = cons.tile([P, N_TILES, N_KS], F32)
            s1 = cons.tile([P, N_TILES], F32)
            lv_all = cons.tile([P, N_TILES], F32)

            ys_t = {}
            r_t = {}

            def group_front(gi):
                """DVE: multiply + in-place tree + alpha-add + clamp;
                ScalarE: square/sqrt (consumed one group later)."""
                gt = GROUP_TILES[gi]
                t0 = GROUP_START[gi]
                m = mid.tile([P, gt, N_KS, DIM], BF16, tag=f"m{gt}")
                for tg, tpi in load_plan[gi]:
                    g = g_tiles.pop((gi, tg))
                    g4 = g[:].rearrange("p c (w d) -> p c w d", d=DIM)
                    nc.vector.tensor_tensor(
                        out=m[:, tg:tg + tpi],
                        in0=g4[:, :, 1:, :],
                        in1=g4[:, :, 0:1, :].to_broadcast([P, tpi, N_KS, DIM]),
                        op=OP.mult,
                    )
                nc.vector.tensor_tensor(
                    out=m[:, :, :, 0:16], in0=m[:, :, :, 0:16],
                    in1=m[:, :, :, 16:32], op=OP.add,
                )
                nc.vector.tensor_tensor(
                    out=m[:, :, :, 0:8], in0=m[:, :, :, 0:8],
                    in1=m[:, :, :, 8:16], op=OP.add,
                )
                nc.vector.tensor_tensor(
                    out=m[:, :, :, 0:4], in0=m[:, :, :, 0:4],
                    in1=m[:, :, :, 4:8], op=OP.add,
                )
                nc.vector.tensor_tensor(
                    out=m[:, :, :, 0:2], in0=m[:, :, :, 0:2],
                    in1=m[:, :, :, 2:4], op=OP.add,
                )
                ys = small.tile([P, gt, N_KS], F32, tag="ys")
                nc.vector.tensor_tensor(
                    out=ys[:], in0=m[:, :, :, 0], in1=m[:, :, :, 1], op=OP.add,
                )
                nc.vector.tensor_tensor(
                    out=ys[:],
                    in0=ys[:],
                    in1=alpha_sb[:, t0:t0 + gt].rearrange(
                        "p (g o) -> p g o", o=1
                    ).to_broadcast([P, gt, N_KS]),
                    op=OP.add,
                )
                # clamp: reference maps d<=1 -> 1+1e-6, i.e. y<=0 -> 1e-6;
                # max(y, 1e-6) differs only for y in (0, 1e-6): measure-zero.
                nc.vector.tensor_scalar(
                    out=ys[:], in0=ys[:], scalar1=1e-6, scalar2=None, op0=OP.max
                )
                sq = small.tile([P, gt, N_KS], F32, tag="sq")
                nc.scalar.activation(
                    out=sq[:], in_=ys[:], func=AF.Square, bias=bias_pos1[:]
                )
                r = small.tile([P, gt, N_KS], F32, tag="r")
                nc.scalar.activation(
                    out=r[:], in_=sq[:], func=AF.Sqrt, bias=bias_neg1[:]
                )
                ys_t[gi] = ys
                r_t[gi] = r

            def group_back(gi):
                """t = (1+ym) + r;  1/t;  row-sum."""
                gt = GROUP_TILES[gi]
                t0 = GROUP_START[gi]
                tg_ = t_all[:, t0:t0 + gt]
                nc.vector.scalar_tensor_tensor(
                    out=tg_, in0=ys_t.pop(gi)[:], scalar=1.0,
                    in1=r_t.pop(gi)[:], op0=OP.add, op1=OP.add,
                )
                rec = small.tile([P, gt, N_KS], F32, tag="rec")
                nc.vector.reciprocal_approx_fast(out=rec[:].opt(), in_=tg_.opt())
                nc.vector.tensor_reduce(
                    out=s1[:, t0:t0 + gt], in_=rec[:],
                    axis=mybir.AxisListType.X, op=OP.add,
                )

            def endgame(lo, hi, part):
                """loss[lo:hi] = ln(t0 * (s1 + 1e-6)); transpose + store."""
                n = hi - lo
                nc.vector.scalar_tensor_tensor(
                    out=s1[:, lo:hi], in0=s1[:, lo:hi], scalar=1e-6,
                    in1=t_all[:, lo:hi, 0], op0=OP.add, op1=OP.mult,
                )
                nc.scalar.activation(
                    out=lv_all[:, lo:hi], in_=s1[:, lo:hi], func=AF.Ln
                )
                lv_t_ps = psum.tile([n, P], F32, space="PSUM", tag=f"ps{part}")
                nc.tensor.transpose(
                    out=lv_t_ps[:], in_=lv_all[:, lo:hi], identity=ident[:]
                )
                lv_t = cons.tile([n, P], F32, tag=f"lvt{part}")
                nc.vector.tensor_copy(out=lv_t[:], in_=lv_t_ps[:])
                nc.sync.dma_start(
                    out=loss[lo * P:hi * P].rearrange("(t p) -> t p", p=P),
                    in_=lv_t[:],
                )

            for gi in range(N_GRP):
                if gi + 2 < N_GRP:
                    for tg, tpi in load_plan[gi + 2]:
                        issue_load(gi + 2, tg, tpi)
                group_front(gi)
                if gi > 0:
                    group_back(gi - 1)
                if gi == N_GRP - 1:
                    endgame(0, GROUP_START[gi], 0)
            group_back(N_GRP - 1)
            endgame(GROUP_START[N_GRP - 1], N_TILES, 1)
    nc.compile()
    return nc


def _get_nc():
    global _nc_cache
    if _nc_cache is None:
        _nc_cache = _build()
    return _nc_cache


def _prep_in_maps(table, I, Ks):
    import ml_dtypes

    table = np.ascontiguousarray(np.asarray(table, dtype=np.float32))
    I = np.asarray(I).astype(np.int64)
    Ks = np.asarray(Ks).astype(np.int64)
    assert table.shape == (N_ITEMS_P1, DIM)
    assert I.shape == (B,) and Ks.shape == (B, N_KS)
    ik = np.concatenate([I[:, None], Ks], axis=1)       # [B, 51]
    rows = table[ik.reshape(-1)].reshape(B, W, DIM)     # [B, 51, 32] fp32
    pack = np.empty((B, W, DIM), dtype=ml_dtypes.bfloat16)
    pack[:, 1:, 0] = rows[:, 1:, 0] - 1.0               # beta_k
    pack[:, 1:, 1:] = rows[:, 1:, 1:]                   # s_k
    pack[:, 0, 0] = 1.0
    pack[:, 0, 1:] = -rows[:, 0, 1:]                    # -s_i
    alpha = (rows[:, 0, 0] - 1.0).astype(np.float32)    # [B]
    g_full = pack.reshape(B, W * DIM)
    in_maps = []
    for c in range(N_CORES):
        sh = np.ascontiguousarray(g_full[c * B_SHARD:(c + 1) * B_SHARD])
        al = np.ascontiguousarray(
            alpha[c * B_SHARD:(c + 1) * B_SHARD].reshape(N_TILES, P).T
        )
        in_maps.append({"g": sh, "alpha": al})
    return in_maps


def _run(table, I, Ks, trace=False, **kwargs):
    from concourse.bass_utils import run_bass_kernel_spmd

    nc = _get_nc()
    in_maps = _prep_in_maps(table, I, Ks)
    res = run_bass_kernel_spmd(
        nc, in_maps, list(range(N_CORES)), trace=trace, **kwargs
    )
    out = np.concatenate(
        [np.asarray(res.results[c]["loss"]) for c in range(N_CORES)]
    ).astype(np.float32)
    return out, res


def kernel(table, I, Ks):
    out, _ = _run(table, I, Ks, trace=False)
    return out
